# revision 3
# baseline (speedup 1.0000x reference)
"""Linear (kernelized) attention for Trainium2, data-parallel over batch N=8
across 8 NeuronCores.

Math (per batch n, head h):
  K' = elu(K)+1, Q' = elu(Q)+1          [S,D] / [L,D]
  KV = K'^T @ V                         [D,D]   (the /S and *S of the
  ksum = sum_s K'                       [D]      reference cancel exactly)
  den[l] = Q'[l,:] . ksum               [L]
  out[l,v] = (Q'[l,:] @ KV)[v] / den[l] [L,D]
eps=1e-6 in the reference is far below one ulp of den (~1e5), so 1/(den+eps)
== 1/den bitwise in fp32.

elu1(x) = exp(min(x,0)) + relu(x), spread across three engines:
  m = min(x,0)        Pool (gpsimd)
  e = Exp(m)          Act  (scalar)
  out = max(x,0)+e    DVE  (vector, bf16 out)

This version is memory-roofline oriented: the per-core HBM traffic (Q,K,V
reads + O write = 32 MiB fp32) is irreducible, so all compute is shrunk
under the DMA stream.  All matmul operands are bf16 (1 PE cycle/row vs 4
for fp32; PSUM accumulation stays fp32, rel-err ~3e-3 vs the 2e-2 gate).

Layout: 4 heads x 32 dims are packed on 128 partitions ("head group"
g in {0,1}).  The main loop interleaves, per 1024-row supertile t:
K/V/Q loads, elu on K and Q, phase-1 accumulation KVfull_g = K'_g^T @
[V_g | 1] into PSUM (KV cols 0:128, ksum col 128), and PE transposes of
the elu'd Q subtiles into qt (bf16, all 64 kept in SBUF).  After the last
K matmul a block-diagonal rhs2_g [128(h,d), 132] = [BD(KV) | ksum columns]
is built in bf16, and the tail loop does one matmul per (subtile, group)
-> [128(l), 132] = [numerator | denominator], a reciprocal plus one
broadcast multiply, and streams O back.  The last supertile's Q prep is
emitted AFTER tails 0..6 so tail-0 matmuls aren't queued behind it on the
in-order PE stream.

V is loaded with a single DMA into an fp32 staging tile and cast on Pool
into the bf16 [V_g | 1] matmul layout (ones columns via tiny memsets).
"""

import os
from contextlib import ExitStack

import numpy as np

N, L, S, H, D = 8, 8192, 8192, 8, 32
HD = H * D  # 256
P = 128
NCORES = 8
NG = 2  # head groups of 4 heads * 32 dim = 128 partitions
GH = 4  # heads per group
TS = int(os.environ.get("KTS", "8"))  # row-tiles per supertile / DMA
KB = int(os.environ.get("KBUF", "2"))  # default buffer depth
QB = int(os.environ.get("KQB", "2"))  # q io bufs
HF = 4  # phase-2 po half-supertile (PSUM bank budget)

_CACHE = {}


def emit_mixattention(ctx, tc, o_ap, q_ap, k_ap, v_ap, L_=L, S_=S, repeat=1, phases="12"):
    from concourse import mybir
    from concourse.masks import make_identity

    nc = tc.nc
    bf16 = mybir.dt.bfloat16

    consts = ctx.enter_context(tc.tile_pool(name="consts", bufs=1))
    io_pool = ctx.enter_context(tc.tile_pool(name="io", bufs=3))
    elw_pool = ctx.enter_context(tc.tile_pool(name="elw", bufs=2))
    qt_pool = ctx.enter_context(tc.tile_pool(name="qt", bufs=6))
    out_pool = ctx.enter_context(tc.tile_pool(name="outp", bufs=3))
    rhs2_pool = ctx.enter_context(tc.tile_pool(name="rhs2", bufs=1))
    small_pool = ctx.enter_context(tc.tile_pool(name="small", bufs=4))
    ps_acc = ctx.enter_context(tc.tile_pool(name="ps_acc", bufs=1, space="PSUM"))
    ps_t = ctx.enter_context(tc.tile_pool(name="ps_t", bufs=2, space="PSUM"))
    ps_o = ctx.enter_context(tc.tile_pool(name="ps_o", bufs=2, space="PSUM"))

    identity = consts.tile([P, P], bf16)
    make_identity(nc, identity)

    pools = (io_pool, elw_pool, qt_pool, out_pool, rhs2_pool, small_pool,
             ps_acc, ps_t, ps_o)

    def _body():
        _emit_body(tc, o_ap, q_ap, k_ap, v_ap, L_, S_, identity, phases, *pools)

    if repeat == 1:
        _body()
    else:
        with tc.For_i(0, repeat, 1):
            _body()


def _emit_body(tc, o_ap, q_ap, k_ap, v_ap, L_, S_, identity, phases,
               io_pool, elw_pool, qt_pool, out_pool, rhs2_pool, small_pool,
               ps_acc, ps_t, ps_o):
    from concourse import mybir

    nc = tc.nc
    f32 = mybir.dt.float32
    bf16 = mybir.dt.bfloat16
    ts = min(TS, S_ // P, L_ // P)  # subtiles per supertile
    hf = min(HF, ts)
    SROWS = ts * P  # rows per supertile
    NST = S_ // SROWS  # number of K/V supertiles
    NLT = L_ // SROWS  # number of Q/O supertiles

    def super_ap(dram, t):
        """[128, ts, HD] view of DRAM rows t*SROWS..(t+1)*SROWS"""
        return dram[t * SROWS:(t + 1) * SROWS, :].rearrange(
            "(c p) d -> p c d", p=P)

    def elu1(x_tile, tag):
        """elu(x)+1 = max(x,0) + exp(min(x,0)) on a [P,ts,HD] supertile.
        min on Pool, exp on Act, combine on DVE; bf16 out for the PE."""
        m = elw_pool.tile([P, ts, HD], f32, tag="m", name="m", bufs=KB)
        nc.gpsimd.tensor_scalar_min(m, x_tile, 0.0)
        e = elw_pool.tile([P, ts, HD], f32, tag="e", name="e", bufs=KB)
        nc.scalar.activation(out=e, in_=m,
                             func=mybir.ActivationFunctionType.Exp, scale=1.0)
        xp = elw_pool.tile([P, ts, HD], bf16, tag=tag, name=tag, bufs=KB)
        nc.vector.scalar_tensor_tensor(out=xp, in0=x_tile, scalar=0.0, in1=e,
                                       op0=mybir.AluOpType.max,
                                       op1=mybir.AluOpType.add)
        return xp

    # ---------------- Phase 1: KV + ksum accumulation -----------------------
    # vtile subtile layout [V_g0 | 1 | V_g1 | 1] (258 cols, bf16) so that
    # rhs_g = vtile[:, c, g*129:(g+1)*129] = [V_g | ones] is contiguous and a
    # single matmul per (c, g) accumulates both KV (cols 0:128) and ksum
    # (col 128) into acc_g [128, 129].
    VW = P + 1  # 129
    acc = [ps_acc.tile([P, NG * VW], f32, tag=f"acc{g}", name=f"acc{g}")
           for g in range(NG)]

    def ph1_super(i):
        ktile = io_pool.tile([P, ts, HD], f32, tag="ktile", name="ktile", bufs=KB)
        nc.gpsimd.dma_start(out=ktile, in_=super_ap(k_ap, i))
        vstage = io_pool.tile([P, ts, HD], f32, tag="vstage", name="vstage", bufs=KB)
        nc.sync.dma_start(out=vstage, in_=super_ap(v_ap, i))
        vtile = elw_pool.tile([P, ts, NG * VW], bf16, tag="vtile", name="vtile", bufs=KB)
        nc.gpsimd.memset(vtile[:, :, P:P + 1], 1.0)
        nc.gpsimd.memset(vtile[:, :, VW + P:VW + P + 1], 1.0)
        nc.gpsimd.tensor_copy(
            out=vtile.rearrange("p c (b w) -> p c b w", b=NG)[:, :, :, 0:P],
            in_=vstage.rearrange("p c (b w) -> p c b w", b=NG))
        kp = elu1(ktile, "kp")
        for c in range(ts):
            first = (i == 0 and c == 0)
            last = (i == NST - 1 and c == ts - 1)
            for g in range(NG):
                nc.tensor.matmul(acc[g][:, 0:VW], kp[:, c, g * P:(g + 1) * P],
                                 vtile[:, c, g * VW:(g + 1) * VW],
                                 start=first, stop=last)

    def build_rhs2():
        # KV block-diagonal copies on Act, ksum column copies on DVE — two
        # engines so the rhs2 critical path (gates every tail matmul) is
        # ~1.5us, emitted before the last supertile's Q-side Act/DVE work.
        rhs2 = []
        for g in range(NG):
            r2 = rhs2_pool.tile([P, 132], bf16, tag=f"rhs2_{g}", name=f"rhs2_{g}")
            nc.vector.memset(r2, 0.0)
            for h in range(GH):
                sl = slice(h * D, (h + 1) * D)
                nc.scalar.copy(out=r2[sl, sl],
                               in_=acc[g][sl, h * D:(h + 1) * D])
                nc.vector.tensor_copy(out=r2[sl, P + h:P + h + 1],
                                      in_=acc[g][sl, P:P + 1])
            rhs2.append(r2)
        return rhs2

    def qprep_super(j, qt_bufs):
        qtile = io_pool.tile([P, ts, HD], f32, tag="qtile", name="qtile",
                             bufs=QB)
        nc.gpsimd.dma_start(out=qtile, in_=super_ap(q_ap, j))
        qp = elu1(qtile, "qp")
        qts = []
        for c in range(ts):
            tp = ps_t.tile([P, NG, P], bf16, tag="tp", name="tp")
            for g in range(NG):
                # both transposes share one PSUM bank: only the first may
                # carry start=True (start zeroes the whole 2KB zero-region)
                nc.tensor.matmul(tp[:, g, :], qp[:, c, g * P:(g + 1) * P],
                                 identity, is_transpose=True,
                                 start=(g == 0), stop=(g == NG - 1))
            qt = qt_pool.tile([P, NG, P], bf16, tag="qt", name="qt",
                              bufs=qt_bufs)
            nc.scalar.copy(out=qt, in_=tp)
            qts.append(qt)
        return qts

    def tail_super(j, qts, rhs2):
        ot = out_pool.tile([P, ts, HD], f32, tag="ot", name="ot", bufs=KB)
        for g in range(NG):
            for hb in range(0, ts, hf):
                # [128, hf, 256]: per subtile 1KB -> no PSUM bank straddle
                po = ps_o.tile([P, hf, HD], f32, tag="po", name="po")
                for ci in range(hf):
                    c = hb + ci
                    # subtiles ci, ci+1 share a PSUM bank: start on even ci
                    nc.tensor.matmul(po[:, ci, 0:132], qts[c][:, g, :],
                                     rhs2[g],
                                     start=(ci % 2 == 0), stop=(ci % 2 == 1))
                rden = small_pool.tile([P, hf, GH], f32, tag="rden",
                                       name="rden")
                nc.vector.reciprocal(rden, po[:, :, P:P + GH])
                num = po[:, :, 0:P].rearrange("p c (h v) -> p c h v", h=GH)
                dst = ot[:, hb:hb + hf, g * P:(g + 1) * P].rearrange(
                    "p c (h v) -> p c h v", h=GH)
                rb = rden[:, :, :].unsqueeze(3).broadcast_to((P, hf, GH, D))
                nc.vector.tensor_mul(out=dst, in0=num, in1=rb)
        nc.sync.dma_start(out=super_ap(o_ap, j), in_=ot)

    if "1" not in phases:
        for g in range(NG):
            nc.vector.memset(acc[g], 1.0)

    QT_BUFS = NLT * ts  # all 64 transposed Q subtiles stay resident (bf16)
    if phases == "12" and NST == NLT:
        preps = {}
        for t in range(NST):
            ph1_super(t)
            if t < NST - 1:
                preps[t] = qprep_super(t, qt_bufs=QT_BUFS)
        rhs2 = build_rhs2()
        for t in range(NLT - 1):
            tail_super(t, preps[t], rhs2)
        preps[NST - 1] = qprep_super(NST - 1, qt_bufs=QT_BUFS)
        tail_super(NLT - 1, preps[NST - 1], rhs2)
    else:
        for i in range(NST if "1" in phases else 0):
            ph1_super(i)
        rhs2 = build_rhs2()
        if "2" not in phases:
            nc.sync.dma_start(out=o_ap[0:P, 0:132], in_=rhs2[0])
        for j in range(NLT if "2" in phases else 0):
            qts = qprep_super(j, qt_bufs=QT_BUFS)
            tail_super(j, qts, rhs2)


def _build(L_=L, S_=S, repeat=1, phases="12"):
    import concourse.bacc as bacc
    import concourse.tile as tile
    from concourse import mybir

    nc = bacc.Bacc("TRN2", target_bir_lowering=False, debug=False,
                   num_devices=NCORES)
    f32 = mybir.dt.float32
    q = nc.dram_tensor("q", [L_, HD], f32, kind="ExternalInput").ap()
    k = nc.dram_tensor("k", [S_, HD], f32, kind="ExternalInput").ap()
    v = nc.dram_tensor("v", [S_, HD], f32, kind="ExternalInput").ap()
    o = nc.dram_tensor("o", [L_, HD], f32, kind="ExternalOutput").ap()
    with tile.TileContext(nc) as tc:
        with ExitStack() as ctx:
            emit_mixattention(ctx, tc, o, q, k, v, L_, S_, repeat=repeat, phases=phases)
    nc.compile()
    return nc


def kernel(queries, keys, values):
    from concourse.bass_utils import run_bass_kernel_spmd

    if "nc" not in _CACHE:
        _CACHE["nc"] = _build()
    nc = _CACHE["nc"]

    in_maps = []
    for i in range(NCORES):
        in_maps.append({
            "q": np.ascontiguousarray(np.asarray(queries[i], np.float32).reshape(L, HD)),
            "k": np.ascontiguousarray(np.asarray(keys[i], np.float32).reshape(S, HD)),
            "v": np.ascontiguousarray(np.asarray(values[i], np.float32).reshape(S, HD)),
        })
    res = run_bass_kernel_spmd(nc, in_maps, core_ids=list(range(NCORES)),
                               trace=os.environ.get("BASS_KERNEL_TRACE", "0") == "1")
    _CACHE["last_result"] = res
    out = np.stack([res.results[i]["o"].reshape(L, H, D) for i in range(NCORES)])
    return out


# revision 5
# speedup vs baseline: 4.8663x; 4.8663x over previous
"""Linear (kernelized) attention for Trainium2, data-parallel over batch N=8
across 8 NeuronCores.

Math (per batch n, head h):
  K' = elu(K)+1, Q' = elu(Q)+1          [S,D] / [L,D]
  KV = K'^T @ V                         [D,D]   (the /S and *S of the
  ksum = sum_s K'                       [D]      reference cancel exactly)
  den[l] = Q'[l,:] . ksum               [L]
  out[l,v] = (Q'[l,:] @ KV)[v] / den[l] [L,D]
eps=1e-6 in the reference is far below one ulp of den (~1e5), so 1/(den+eps)
== 1/den bitwise in fp32.

Memory-roofline orientation: per-core HBM traffic (Q,K,V reads + O write =
32 MiB fp32) is irreducible at ~330 GB/s/core -> ~100 us floor, so ALL
compute must hide under the DMA stream.  Profiling showed the PE is the
binding engine when any matmul path is fp32 (4 cycles/row + un-FWL'd
207 ns LDWEIGHTS per 128-wide matmul), so every matmul operand is bf16
(1 cycle/row, FWL halves weight loads; PSUM accumulation stays fp32;
measured rel-err ~2.7e-3 vs the 2e-2 gate).

Engine placement per 1024-row supertile (DMA slot ~9.5 us), chosen from
measured per-engine rates (Act 0.83 ns/elem, DVE 1.04 fp32 / 0.5 bf16,
GpSimd ~4.2 ns/elem, and a 4x DVE penalty writing 16-bit from fp32 which
rules DVE out for casts):
  Act:    kb=cast(K), ke=Exp(km), qb=cast(Q), qe=Exp(qm), 3/8 of V cast
  DVE:    km=min(kb,0), kp=max(kb,0)+ke, same for Q (all-bf16 2x mode),
          8 transposed-Q PSUM->SBUF drains (bf16)
  GpSimd: K/Q DMA triggers, ones memsets, 5/8 of V cast
  Sync:   V load (one DMA), O store
  PE:     phase-1 accumulation, Q transposes, tail matmuls (all bf16)

Layout: 4 heads x 32 dims are packed on 128 partitions ("head group"
g in {0,1}).  vtile subtile layout [V_g0 | 1 | V_g1 | 1] (bf16) lets one
matmul per (subtile, group) accumulate KV (cols 0:128) and ksum (col 128)
into PSUM acc_g.  All 64 transposed bf16 Q subtiles stay resident in SBUF
(32 KB/partition).  rhs2_g [128(h,d), 132] = [BD(KV) | ksum columns] is
built in bf16 right after the last phase-1 matmul, and tails 0..6 are
emitted BEFORE the last supertile's Q prep so tail-0 matmuls aren't queued
behind it on the in-order PE stream.  Tail t: one matmul per (subtile,
group) -> [128(l), 132] = [numerator | denominator], reciprocal +
broadcast multiply (DVE), O store.
"""

import os
from contextlib import ExitStack

import numpy as np

N, L, S, H, D = 8, 8192, 8192, 8, 32
HD = H * D  # 256
P = 128
NCORES = 8
NG = 2  # head groups of 4 heads * 32 dim = 128 partitions
GH = 4  # heads per group
TS = int(os.environ.get("KTS", "8"))  # row-tiles per supertile / DMA
KB = int(os.environ.get("KBUF", "2"))  # default buffer depth
QB = int(os.environ.get("KQB", "2"))  # q io bufs
HF = 4  # phase-2 po half-supertile (PSUM bank budget)
VC = int(os.environ.get("KVC", "5"))  # V-cast subtiles on GpSimd (rest: Act)

_CACHE = {}


def emit_mixattention(ctx, tc, o_ap, q_ap, k_ap, v_ap, L_=L, S_=S, repeat=1, phases="12"):
    from concourse import mybir
    from concourse.masks import make_identity

    nc = tc.nc
    bf16 = mybir.dt.bfloat16

    consts = ctx.enter_context(tc.tile_pool(name="consts", bufs=1))
    io_pool = ctx.enter_context(tc.tile_pool(name="io", bufs=3))
    elw_pool = ctx.enter_context(tc.tile_pool(name="elw", bufs=2))
    qt_pool = ctx.enter_context(tc.tile_pool(name="qt", bufs=6))
    out_pool = ctx.enter_context(tc.tile_pool(name="outp", bufs=3))
    rhs2_pool = ctx.enter_context(tc.tile_pool(name="rhs2", bufs=1))
    small_pool = ctx.enter_context(tc.tile_pool(name="small", bufs=4))
    ps_acc = ctx.enter_context(tc.tile_pool(name="ps_acc", bufs=1, space="PSUM"))
    ps_t = ctx.enter_context(tc.tile_pool(name="ps_t", bufs=2, space="PSUM"))
    ps_o = ctx.enter_context(tc.tile_pool(name="ps_o", bufs=2, space="PSUM"))

    identity = consts.tile([P, P], bf16)
    make_identity(nc, identity)

    pools = (io_pool, elw_pool, qt_pool, out_pool, rhs2_pool, small_pool,
             ps_acc, ps_t, ps_o)

    def _body():
        _emit_body(tc, o_ap, q_ap, k_ap, v_ap, L_, S_, identity, phases, *pools)

    if repeat == 1:
        _body()
    else:
        with tc.For_i(0, repeat, 1):
            _body()


def _emit_body(tc, o_ap, q_ap, k_ap, v_ap, L_, S_, identity, phases,
               io_pool, elw_pool, qt_pool, out_pool, rhs2_pool, small_pool,
               ps_acc, ps_t, ps_o):
    from concourse import mybir

    nc = tc.nc
    f32 = mybir.dt.float32
    bf16 = mybir.dt.bfloat16
    ts = min(TS, S_ // P, L_ // P)  # subtiles per supertile
    hf = min(HF, ts)
    vc = min(VC, ts)
    SROWS = ts * P  # rows per supertile
    NST = S_ // SROWS  # number of K/V supertiles
    NLT = L_ // SROWS  # number of Q/O supertiles

    def super_ap(dram, t):
        """[128, ts, HD] view of DRAM rows t*SROWS..(t+1)*SROWS"""
        return dram[t * SROWS:(t + 1) * SROWS, :].rearrange(
            "(c p) d -> p c d", p=P)

    def elu1(x_tile, tag):
        """elu(x)+1 = max(x,0) + exp(min(x,0)), all bf16 after one Act cast"""
        xb = elw_pool.tile([P, ts, HD], bf16, tag="xb", name="xb", bufs=KB)
        nc.scalar.copy(out=xb, in_=x_tile)
        m = elw_pool.tile([P, ts, HD], bf16, tag="m", name="m", bufs=KB)
        nc.vector.tensor_scalar_min(m, xb, 0.0)
        e = elw_pool.tile([P, ts, HD], bf16, tag="e", name="e", bufs=KB)
        nc.scalar.activation(out=e, in_=m,
                             func=mybir.ActivationFunctionType.Exp, scale=1.0)
        xp = elw_pool.tile([P, ts, HD], bf16, tag=tag, name=tag, bufs=KB)
        nc.vector.scalar_tensor_tensor(out=xp, in0=xb, scalar=0.0, in1=e,
                                       op0=mybir.AluOpType.max,
                                       op1=mybir.AluOpType.add)
        return xp

    # ---------------- Phase 1: KV + ksum accumulation -----------------------
    VW = P + 1  # 129
    acc = [ps_acc.tile([P, NG * VW], f32, tag=f"acc{g}", name=f"acc{g}")
           for g in range(NG)]

    def ph1_super(i):
        ktile = io_pool.tile([P, ts, HD], f32, tag="ktile", name="ktile", bufs=KB)
        nc.gpsimd.dma_start(out=ktile, in_=super_ap(k_ap, i))
        vstage = io_pool.tile([P, ts, HD], f32, tag="vstage", name="vstage", bufs=KB)
        nc.sync.dma_start(out=vstage, in_=super_ap(v_ap, i))
        vtile = elw_pool.tile([P, ts, NG * VW], bf16, tag="vtile", name="vtile", bufs=KB)
        nc.gpsimd.memset(vtile[:, :, P:P + 1], 1.0)
        nc.gpsimd.memset(vtile[:, :, VW + P:VW + P + 1], 1.0)
        # fp32 -> bf16 V cast, split GpSimd (slow but idle) / Act (fast, busy)
        vdst = vtile.rearrange("p c (b w) -> p c b w", b=NG)[:, :, :, 0:P]
        vsrc = vstage.rearrange("p c (b w) -> p c b w", b=NG)
        if vc > 0:
            nc.gpsimd.tensor_copy(out=vdst[:, 0:vc], in_=vsrc[:, 0:vc])
        if vc < ts:
            nc.scalar.copy(out=vdst[:, vc:ts], in_=vsrc[:, vc:ts])
        kp = elu1(ktile, "kp")
        for c in range(ts):
            first = (i == 0 and c == 0)
            last = (i == NST - 1 and c == ts - 1)
            for g in range(NG):
                nc.tensor.matmul(acc[g][:, 0:VW], kp[:, c, g * P:(g + 1) * P],
                                 vtile[:, c, g * VW:(g + 1) * VW],
                                 start=first, stop=last)

    def build_rhs2():
        # KV block-diagonal copies on Act, ksum column copies on DVE — two
        # engines so the rhs2 critical path (gates every tail matmul) stays
        # short; emitted before the last supertile's Q-side Act/DVE work.
        rhs2 = []
        for g in range(NG):
            r2 = rhs2_pool.tile([P, 132], bf16, tag=f"rhs2_{g}", name=f"rhs2_{g}")
            nc.vector.memset(r2, 0.0)
            for h in range(GH):
                sl = slice(h * D, (h + 1) * D)
                nc.scalar.copy(out=r2[sl, sl],
                               in_=acc[g][sl, h * D:(h + 1) * D])
                nc.vector.tensor_copy(out=r2[sl, P + h:P + h + 1],
                                      in_=acc[g][sl, P:P + 1])
            rhs2.append(r2)
        return rhs2

    def qprep_super(j, qt_bufs):
        qtile = io_pool.tile([P, ts, HD], f32, tag="qtile", name="qtile",
                             bufs=QB)
        nc.gpsimd.dma_start(out=qtile, in_=super_ap(q_ap, j))
        qp = elu1(qtile, "qp")
        qts = []
        for c in range(ts):
            tp = ps_t.tile([P, NG, P], bf16, tag="tp", name="tp")
            for g in range(NG):
                # both transposes share one PSUM bank: only the first may
                # carry start=True (start zeroes the whole 2KB zero-region)
                nc.tensor.matmul(tp[:, g, :], qp[:, c, g * P:(g + 1) * P],
                                 identity, is_transpose=True,
                                 start=(g == 0), stop=(g == NG - 1))
            qt = qt_pool.tile([P, NG, P], bf16, tag="qt", name="qt",
                              bufs=qt_bufs)
            nc.vector.tensor_copy(out=qt, in_=tp)
            qts.append(qt)
        return qts

    def tail_super(j, qts, rhs2):
        ot = out_pool.tile([P, ts, HD], f32, tag="ot", name="ot", bufs=KB)
        for g in range(NG):
            for hb in range(0, ts, hf):
                # [128, hf, 256]: per subtile 1KB -> no PSUM bank straddle
                po = ps_o.tile([P, hf, HD], f32, tag="po", name="po")
                for ci in range(hf):
                    c = hb + ci
                    # subtiles ci, ci+1 share a PSUM bank: start on even ci
                    nc.tensor.matmul(po[:, ci, 0:132], qts[c][:, g, :],
                                     rhs2[g],
                                     start=(ci % 2 == 0), stop=(ci % 2 == 1))
                rden = small_pool.tile([P, hf, GH], f32, tag="rden",
                                       name="rden")
                nc.vector.reciprocal(rden, po[:, :, P:P + GH])
                num = po[:, :, 0:P].rearrange("p c (h v) -> p c h v", h=GH)
                dst = ot[:, hb:hb + hf, g * P:(g + 1) * P].rearrange(
                    "p c (h v) -> p c h v", h=GH)
                rb = rden[:, :, :].unsqueeze(3).broadcast_to((P, hf, GH, D))
                nc.vector.tensor_mul(out=dst, in0=num, in1=rb)
        nc.sync.dma_start(out=super_ap(o_ap, j), in_=ot)

    if "1" not in phases:
        for g in range(NG):
            nc.vector.memset(acc[g], 1.0)

    QT_BUFS = NLT * ts  # all 64 transposed Q subtiles stay resident (bf16)
    if phases == "12" and NST == NLT:
        preps = {}
        for t in range(NST):
            ph1_super(t)
            if t < NST - 1:
                preps[t] = qprep_super(t, qt_bufs=QT_BUFS)
        rhs2 = build_rhs2()
        for t in range(NLT - 1):
            tail_super(t, preps[t], rhs2)
        preps[NST - 1] = qprep_super(NST - 1, qt_bufs=QT_BUFS)
        tail_super(NLT - 1, preps[NST - 1], rhs2)
    else:
        for i in range(NST if "1" in phases else 0):
            ph1_super(i)
        rhs2 = build_rhs2()
        if "2" not in phases:
            nc.sync.dma_start(out=o_ap[0:P, 0:132], in_=rhs2[0])
        for j in range(NLT if "2" in phases else 0):
            qts = qprep_super(j, qt_bufs=QT_BUFS)
            tail_super(j, qts, rhs2)


def _build(L_=L, S_=S, repeat=1, phases="12"):
    import concourse.bacc as bacc
    import concourse.tile as tile
    from concourse import mybir

    nc = bacc.Bacc("TRN2", target_bir_lowering=False, debug=False,
                   num_devices=NCORES)
    f32 = mybir.dt.float32
    q = nc.dram_tensor("q", [L_, HD], f32, kind="ExternalInput").ap()
    k = nc.dram_tensor("k", [S_, HD], f32, kind="ExternalInput").ap()
    v = nc.dram_tensor("v", [S_, HD], f32, kind="ExternalInput").ap()
    o = nc.dram_tensor("o", [L_, HD], f32, kind="ExternalOutput").ap()
    with tile.TileContext(nc) as tc:
        with ExitStack() as ctx:
            emit_mixattention(ctx, tc, o, q, k, v, L_, S_, repeat=repeat, phases=phases)
    nc.compile()
    return nc


def kernel(queries, keys, values):
    from concourse.bass_utils import run_bass_kernel_spmd

    if "nc" not in _CACHE:
        _CACHE["nc"] = _build()
    nc = _CACHE["nc"]

    in_maps = []
    for i in range(NCORES):
        in_maps.append({
            "q": np.ascontiguousarray(np.asarray(queries[i], np.float32).reshape(L, HD)),
            "k": np.ascontiguousarray(np.asarray(keys[i], np.float32).reshape(S, HD)),
            "v": np.ascontiguousarray(np.asarray(values[i], np.float32).reshape(S, HD)),
        })
    res = run_bass_kernel_spmd(nc, in_maps, core_ids=list(range(NCORES)),
                               trace=os.environ.get("BASS_KERNEL_TRACE", "0") == "1")
    _CACHE["last_result"] = res
    out = np.stack([res.results[i]["o"].reshape(L, H, D) for i in range(NCORES)])
    return out
